# revision 1
# baseline (speedup 1.0000x reference)
"""Trainium2 Bass kernel: 3x3 conv (stride 1, pad 1) + bias, NCHW.

x[16,128,112,112] * w[256,128,3,3] + b[256] -> y[16,256,112,112]

Strategy: data-parallel over batch (2 images per core, 8 cores).
Per core the conv is 9 shifted fp32r matmuls accumulated in PSUM:
contraction dim = cin(128) on partitions, stationary = w slice
[cin,128cout], moving = padded-image rows [cin, 4x112]. Bias is fused
into the PSUM->SBUF drain via ScalarE Identity activation.
"""
import numpy as np
from concourse import bacc, mybir
import concourse.tile as tile
from concourse.bass_utils import run_bass_kernel_spmd

F32R = mybir.dt.float32r
F32 = mybir.dt.float32

B, CIN, H, W = 16, 128, 112, 112
COUT = 256
KH = KW = 3
HP = WP = 114          # padded
NCORES = 8
BPC = B // NCORES      # images per core
NR = 4                 # output rows per PSUM tile (free = 448 <= 512)
NCHUNK = 28            # output rows per staged out-chunk / DMA

_cache = {}


def _build():
    nc = bacc.Bacc(None)
    x_d = nc.dram_tensor("xp", [BPC, CIN, HP, WP], F32R, kind="ExternalInput")
    w_d = nc.dram_tensor("wt", [CIN, 2 * 9 * 128], F32R, kind="ExternalInput")
    b_d = nc.dram_tensor("bt", [CIN, 2], F32, kind="ExternalInput")
    y_d = nc.dram_tensor("y", [BPC, COUT, H, W], F32, kind="ExternalOutput")

    with tile.TileContext(nc) as tc:
        with (
            tc.tile_pool(name="xpool", bufs=BPC) as xpool,
            tc.tile_pool(name="wpool", bufs=1) as wpool,
            tc.tile_pool(name="bpool", bufs=1) as bpool,
            tc.tile_pool(name="psum", bufs=8, space="PSUM") as psum,
            tc.tile_pool(name="opool", bufs=3) as opool,
        ):
            w_t = wpool.tile([CIN, 2 * 9 * 128], F32R)
            b_t = bpool.tile([CIN, 2], F32)
            nc.sync.dma_start(w_t[:], w_d[:])
            nc.sync.dma_start(b_t[:], b_d[:])
            x_ts = []
            for img in range(BPC):
                x_t = xpool.tile([CIN, HP, WP], F32R, tag="x")
                nc.sync.dma_start(x_t[:], x_d[img])
                x_ts.append(x_t)

            for img in range(BPC):
                x_t = x_ts[img]
                for cb in range(2):
                    for c0 in range(0, H, NCHUNK):
                        ot = opool.tile([128, NCHUNK, W], F32, tag="o")
                        for r0 in range(c0, c0 + NCHUNK, NR):
                            ps = psum.tile([128, NR, W], F32, tag="ps")
                            k = 0
                            for dy in range(KH):
                                for dx in range(KW):
                                    idx = (cb * 3 + dy) * 3 + dx
                                    nc.tensor.matmul(
                                        ps[:],
                                        w_t[:, idx * 128:(idx + 1) * 128],
                                        x_t[:, r0 + dy:r0 + dy + NR, dx:dx + W],
                                        start=(k == 0),
                                        stop=(k == 8),
                                    )
                                    k += 1
                            nc.scalar.activation(
                                ot[:, r0 - c0:r0 - c0 + NR, :],
                                ps[:],
                                mybir.ActivationFunctionType.Identity,
                                bias=b_t[:, cb:cb + 1],
                            )
                        nc.sync.dma_start(
                            y_d[img, cb * 128:(cb + 1) * 128, c0:c0 + NCHUNK, :],
                            ot[:],
                        )
    nc.compile()
    return nc


def _prep(x, weight, bias):
    x = np.asarray(x, dtype=np.float32)
    weight = np.asarray(weight, dtype=np.float32)
    bias = np.asarray(bias, dtype=np.float32)
    xp = np.pad(x, ((0, 0), (0, 0), (1, 1), (1, 1)))
    # wt[cin, ((cb*3+dy)*3+dx)*128 + co] = weight[cb*128+co, cin, dy, dx]
    wt = np.ascontiguousarray(
        weight.reshape(2, 128, CIN, KH, KW).transpose(2, 0, 3, 4, 1).reshape(CIN, -1)
    )
    bt = np.ascontiguousarray(bias.reshape(2, 128).T)
    in_maps = [
        {
            "xp": np.ascontiguousarray(xp[c * BPC:(c + 1) * BPC]),
            "wt": wt,
            "bt": bt,
        }
        for c in range(NCORES)
    ]
    return in_maps


def _run(x, weight, bias, **spmd_kwargs):
    if "nc" not in _cache:
        _cache["nc"] = _build()
    nc = _cache["nc"]
    in_maps = _prep(x, weight, bias)
    res = run_bass_kernel_spmd(nc, in_maps, list(range(NCORES)), **spmd_kwargs)
    y = np.concatenate([res.results[c]["y"] for c in range(NCORES)], axis=0)
    return y, res


def kernel(x, weight, bias):
    y, _ = _run(x, weight, bias)
    return y


# revision 3
# speedup vs baseline: 1.0162x; 1.0162x over previous
"""Trainium2 Bass kernel: 3x3 conv (stride 1, pad 1) + bias, NCHW.

x[16,128,112,112] * w[256,128,3,3] + b[256] -> y[16,256,112,112]

Strategy: data-parallel over batch (2 images per core, 8 cores).
Per core the conv is 9 shifted fp32r matmuls accumulated in PSUM:
contraction dim = cin(128) on partitions, stationary = w slice
[cin,128cout], moving = padded-image rows [cin, 4x112]. Bias is fused
into the PSUM->SBUF drain via ScalarE Identity activation.

v2: chunked input loads (compute starts ~3us in, not after the full
6.6MB image), image-1 load deferred in program order, stores on the
gpsimd SWDGE ring so they don't queue behind loads, PE warmup matmuls
to absorb the HAM cold-throttle, 16-row out chunks for a short tail.
"""
import numpy as np
from concourse import bacc, mybir
import concourse.tile as tile
from concourse.bass_utils import run_bass_kernel_spmd

F32R = mybir.dt.float32r
F32 = mybir.dt.float32

B, CIN, H, W = 16, 128, 112, 112
COUT = 256
KH = KW = 3
HP = WP = 114          # padded
NCORES = 8
BPC = B // NCORES      # images per core
NR = 4                 # output rows per PSUM tile (free = 448 <= 512)
NCHUNK = 16            # output rows per staged out-chunk / DMA
XCHUNKS = [(0, 8), (8, 23), (23, 38), (38, 53), (53, 68),
           (68, 83), (83, 98), (98, 114)]
NWARM = 28             # PE warmup matmuls (N=64) during head DMA

_cache = {}


def _build():
    nc = bacc.Bacc(None)
    x_d = nc.dram_tensor("xp", [BPC, CIN, HP, WP], F32R, kind="ExternalInput")
    w_d = nc.dram_tensor("wt", [CIN, 2 * 9 * 128], F32R, kind="ExternalInput")
    b_d = nc.dram_tensor("bt", [CIN, 2], F32, kind="ExternalInput")
    y_d = nc.dram_tensor("y", [BPC, COUT, H, W], F32, kind="ExternalOutput")

    with tile.TileContext(nc) as tc:
        with (
            tc.tile_pool(name="xpool", bufs=BPC) as xpool,
            tc.tile_pool(name="wpool", bufs=1) as wpool,
            tc.tile_pool(name="bpool", bufs=1) as bpool,
            tc.tile_pool(name="warms", bufs=1) as warms,
            tc.tile_pool(name="psum", bufs=7, space="PSUM") as psum,
            tc.tile_pool(name="warmp", bufs=1, space="PSUM") as warmp,
            tc.tile_pool(name="opool", bufs=4) as opool,
        ):
            # --- PE warmup: keep the HAM activity window busy while the
            # first input chunks stream in, so real matmuls run at 2.4GHz.
            wsrc = warms.tile([128, 128], F32)
            nc.gpsimd.memset(wsrc[:], 0.0)
            wps = warmp.tile([128, 64], F32)
            for _ in range(NWARM):
                nc.tensor.matmul(wps[:], wsrc[:].bitcast(F32R),
                                 wsrc[:, 0:64].bitcast(F32R),
                                 start=True, stop=True)

            # --- loads (sync engine = one HWDGE FIFO ring, program order):
            # image-0 chunks first, then weights (cb0 half first), bias.
            x_ts = []
            x_t0 = xpool.tile([CIN, HP, WP], F32R, tag="x")
            x_ts.append(x_t0)
            for a, b in XCHUNKS:
                nc.sync.dma_start(x_t0[:, a:b, :], x_d[0, :, a:b, :])
            w_t = wpool.tile([CIN, 2 * 9 * 128], F32R)
            nc.sync.dma_start(w_t[:, :9 * 128], w_d[:, :9 * 128])
            nc.sync.dma_start(w_t[:, 9 * 128:], w_d[:, 9 * 128:])
            b_t = bpool.tile([CIN, 2], F32)
            nc.sync.dma_start(b_t[:], b_d[:])
            x_t1 = xpool.tile([CIN, HP, WP], F32R, tag="x")
            x_ts.append(x_t1)

            def img1_load(c):
                a, b = XCHUNKS[c]
                nc.sync.dma_start(x_t1[:, a:b, :], x_d[1, :, a:b, :])

            for img in range(BPC):
                x_t = x_ts[img]
                for cb in range(2):
                    for ci, c0 in enumerate(range(0, H, NCHUNK)):
                        ot = opool.tile([128, NCHUNK, W], F32, tag="o")
                        for r0 in range(c0, c0 + NCHUNK, NR):
                            ps = psum.tile([128, NR, W], F32, tag="ps")
                            k = 0
                            for dy in range(KH):
                                for dx in range(KW):
                                    idx = (cb * 3 + dy) * 3 + dx
                                    nc.tensor.matmul(
                                        ps[:],
                                        w_t[:, idx * 128:(idx + 1) * 128],
                                        x_t[:, r0 + dy:r0 + dy + NR, dx:dx + W],
                                        start=(k == 0),
                                        stop=(k == 8),
                                    )
                                    k += 1
                            nc.scalar.activation(
                                ot[:, r0 - c0:r0 - c0 + NR, :],
                                ps[:],
                                mybir.ActivationFunctionType.Identity,
                                bias=b_t[:, cb:cb + 1],
                            )
                        nc.gpsimd.dma_start(
                            y_d[img, cb * 128:(cb + 1) * 128, c0:c0 + NCHUNK, :],
                            ot[:],
                        )
                        # defer image-1 chunk loads into image-0/cb0 compute
                        if img == 0 and cb == 0 and ci < len(XCHUNKS):
                            img1_load(ci)
                    if img == 0 and cb == 0:
                        img1_load(7)
    nc.compile()
    return nc


def _prep(x, weight, bias):
    x = np.asarray(x, dtype=np.float32)
    weight = np.asarray(weight, dtype=np.float32)
    bias = np.asarray(bias, dtype=np.float32)
    xp = np.pad(x, ((0, 0), (0, 0), (1, 1), (1, 1)))
    # wt[cin, ((cb*3+dy)*3+dx)*128 + co] = weight[cb*128+co, cin, dy, dx]
    wt = np.ascontiguousarray(
        weight.reshape(2, 128, CIN, KH, KW).transpose(2, 0, 3, 4, 1).reshape(CIN, -1)
    )
    bt = np.ascontiguousarray(bias.reshape(2, 128).T)
    in_maps = [
        {
            "xp": np.ascontiguousarray(xp[c * BPC:(c + 1) * BPC]),
            "wt": wt,
            "bt": bt,
        }
        for c in range(NCORES)
    ]
    return in_maps


def _run(x, weight, bias, **spmd_kwargs):
    if "nc" not in _cache:
        _cache["nc"] = _build()
    nc = _cache["nc"]
    in_maps = _prep(x, weight, bias)
    res = run_bass_kernel_spmd(nc, in_maps, list(range(NCORES)), **spmd_kwargs)
    y = np.concatenate([res.results[c]["y"] for c in range(NCORES)], axis=0)
    return y, res


def kernel(x, weight, bias):
    y, _ = _run(x, weight, bias)
    return y


# revision 6
# speedup vs baseline: 1.0893x; 1.0720x over previous
"""Trainium2 Bass kernel: 3x3 conv (stride 1, pad 1) + bias, NCHW.

x[16,128,112,112] * w[256,128,3,3] + b[256] -> y[16,256,112,112]

Strategy: data-parallel over batch (2 images per core, 8 cores).
Per core the conv is 9 shifted fp32r matmuls accumulated in PSUM:
contraction dim = cin(128) on partitions, stationary = w slice
[cin,128cout], moving = padded-image rows [cin, 4x112]. Bias is fused
into the PSUM->SBUF drain via ScalarE Identity activation.

v2: chunked input loads (compute starts ~3us in, not after the full
6.6MB image), image-1 load deferred in program order, stores on the
gpsimd SWDGE ring so they don't queue behind loads, PE warmup matmuls
to absorb the HAM cold-throttle, 16-row out chunks for a short tail.
"""
import numpy as np
from concourse import bacc, mybir
import concourse.tile as tile
from concourse.bass_utils import run_bass_kernel_spmd

F32R = mybir.dt.float32r
F32 = mybir.dt.float32

B, CIN, H, W = 16, 128, 112, 112
COUT = 256
KH = KW = 3
HP = WP = 114          # padded
NCORES = 8
BPC = B // NCORES      # images per core
NR = 4                 # output rows per PSUM tile (free = 448 <= 512)
NCHUNK = 16            # output rows per staged out-chunk / DMA
XCHUNKS = [(0, 8), (8, 23), (23, 38), (38, 53), (53, 68),
           (68, 83), (83, 98), (98, 114)]
NWARM = 8              # PE warmup matmuls during head DMA (~400ns each)

_cache = {}


def _build():
    nc = bacc.Bacc(None)
    x_d = nc.dram_tensor("xp", [BPC, CIN, HP, WP], F32R, kind="ExternalInput")
    w_d = nc.dram_tensor("wt", [CIN, 2 * 9 * 128], F32R, kind="ExternalInput")
    b_d = nc.dram_tensor("bt", [CIN, 2], F32, kind="ExternalInput")
    y_d = nc.dram_tensor("y", [BPC, COUT, H, W], F32, kind="ExternalOutput")

    with tile.TileContext(nc) as tc:
        with (
            tc.tile_pool(name="xpool", bufs=BPC) as xpool,
            tc.tile_pool(name="wpool", bufs=1) as wpool,
            tc.tile_pool(name="bpool", bufs=1) as bpool,
            tc.tile_pool(name="warms", bufs=1) as warms,
            tc.tile_pool(name="psum", bufs=7, space="PSUM") as psum,
            tc.tile_pool(name="warmp", bufs=1, space="PSUM") as warmp,
            tc.tile_pool(name="opool", bufs=4) as opool,
        ):
            # --- PE warmup: keep the HAM activity window busy while the
            # first input chunks stream in, so real matmuls run at 2.4GHz.
            wsrc = warms.tile([128, 128], F32)
            nc.gpsimd.memset(wsrc[:], 0.0)
            wps = warmp.tile([128, 64], F32)
            for _ in range(NWARM):
                nc.tensor.matmul(wps[:], wsrc[:].bitcast(F32R),
                                 wsrc[:, 0:64].bitcast(F32R),
                                 start=True, stop=True)

            # --- loads (sync engine = one HWDGE FIFO ring, program order):
            # interleave so the first psum group's deps (chunk0 + w-half-0)
            # land first on the FIFO, then the rest.
            x_ts = []
            x_t0 = xpool.tile([CIN, HP, WP], F32R, tag="x")
            x_ts.append(x_t0)
            w_t = wpool.tile([CIN, 2 * 9 * 128], F32R)
            b_t = bpool.tile([CIN, 2], F32)

            def xload(x_t, img, c):
                a, b = XCHUNKS[c]
                nc.sync.dma_start(x_t[:, a:b, :], x_d[img, :, a:b, :])

            xload(x_t0, 0, 0)
            nc.sync.dma_start(w_t[:, :9 * 128], w_d[:, :9 * 128])
            xload(x_t0, 0, 1)
            nc.sync.dma_start(w_t[:, 9 * 128:], w_d[:, 9 * 128:])
            nc.sync.dma_start(b_t[:], b_d[:])
            for c in range(2, len(XCHUNKS)):
                xload(x_t0, 0, c)
            x_t1 = xpool.tile([CIN, HP, WP], F32R, tag="x")
            x_ts.append(x_t1)

            def img1_load(c):
                xload(x_t1, 1, c)

            for img in range(BPC):
                x_t = x_ts[img]
                for cb in range(2):
                    for ci, c0 in enumerate(range(0, H, NCHUNK)):
                        ot = opool.tile([128, NCHUNK, W], F32, tag="o")
                        for r0 in range(c0, c0 + NCHUNK, NR):
                            ps = psum.tile([128, NR, W], F32, tag="ps")
                            k = 0
                            for dy in range(KH):
                                for dx in range(KW):
                                    idx = (cb * 3 + dy) * 3 + dx
                                    nc.tensor.matmul(
                                        ps[:],
                                        w_t[:, idx * 128:(idx + 1) * 128],
                                        x_t[:, r0 + dy:r0 + dy + NR, dx:dx + W],
                                        start=(k == 0),
                                        stop=(k == 8),
                                    )
                                    k += 1
                            nc.scalar.activation(
                                ot[:, r0 - c0:r0 - c0 + NR, :],
                                ps[:],
                                mybir.ActivationFunctionType.Identity,
                                bias=b_t[:, cb:cb + 1],
                            )
                        nc.gpsimd.dma_start(
                            y_d[img, cb * 128:(cb + 1) * 128, c0:c0 + NCHUNK, :],
                            ot[:],
                        )
                        # defer image-1 chunk loads into image-0/cb0 compute
                        if img == 0 and cb == 0 and ci < len(XCHUNKS):
                            img1_load(ci)
                    if img == 0 and cb == 0:
                        img1_load(7)
    nc.compile()
    return nc


def _prep(x, weight, bias):
    x = np.asarray(x, dtype=np.float32)
    weight = np.asarray(weight, dtype=np.float32)
    bias = np.asarray(bias, dtype=np.float32)
    xp = np.pad(x, ((0, 0), (0, 0), (1, 1), (1, 1)))
    # wt[cin, ((cb*3+dy)*3+dx)*128 + co] = weight[cb*128+co, cin, dy, dx]
    wt = np.ascontiguousarray(
        weight.reshape(2, 128, CIN, KH, KW).transpose(2, 0, 3, 4, 1).reshape(CIN, -1)
    )
    bt = np.ascontiguousarray(bias.reshape(2, 128).T)
    in_maps = [
        {
            "xp": np.ascontiguousarray(xp[c * BPC:(c + 1) * BPC]),
            "wt": wt,
            "bt": bt,
        }
        for c in range(NCORES)
    ]
    return in_maps


def _run(x, weight, bias, **spmd_kwargs):
    if "nc" not in _cache:
        _cache["nc"] = _build()
    nc = _cache["nc"]
    in_maps = _prep(x, weight, bias)
    res = run_bass_kernel_spmd(nc, in_maps, list(range(NCORES)), **spmd_kwargs)
    y = np.concatenate([res.results[c]["y"] for c in range(NCORES)], axis=0)
    return y, res


def kernel(x, weight, bias):
    y, _ = _run(x, weight, bias)
    return y


# revision 9
# speedup vs baseline: 1.0918x; 1.0022x over previous
"""Trainium2 Bass kernel: 3x3 conv (stride 1, pad 1) + bias, NCHW.

x[16,128,112,112] * w[256,128,3,3] + b[256] -> y[16,256,112,112]

Strategy: data-parallel over batch (2 images per core, 8 cores).
Per core the conv is 9 shifted fp32r matmuls accumulated in PSUM:
contraction dim = cin(128) on partitions, stationary = w slice
[cin,128cout], moving = padded-image rows [cin, 4x112]. Bias is fused
into the PSUM->SBUF drain via ScalarE Identity activation.

v2: chunked input loads (compute starts ~3us in, not after the full
6.6MB image), image-1 load deferred in program order, stores on the
gpsimd SWDGE ring so they don't queue behind loads, PE warmup matmuls
to absorb the HAM cold-throttle, 16-row out chunks for a short tail.
"""
import numpy as np
from concourse import bacc, mybir
import concourse.tile as tile
from concourse.bass_utils import run_bass_kernel_spmd

F32R = mybir.dt.float32r
F32 = mybir.dt.float32

B, CIN, H, W = 16, 128, 112, 112
COUT = 256
KH = KW = 3
HP = WP = 114          # padded
NCORES = 8
BPC = B // NCORES      # images per core
NR = 4                 # output rows per PSUM tile (free = 448 <= 512)
NCHUNK = 16            # output rows per staged out-chunk / DMA
XCHUNKS = [(0, 6), (6, 22), (22, 38), (38, 54), (54, 70),
           (70, 86), (86, 100), (100, 114)]
NWARM = 8              # PE warmup matmuls during head DMA (~400ns each)

_cache = {}


def _build():
    nc = bacc.Bacc(None)
    x_d = nc.dram_tensor("xp", [BPC, CIN, HP, WP], F32R, kind="ExternalInput")
    w_d = nc.dram_tensor("wt", [CIN, 2 * 9 * 128], F32R, kind="ExternalInput")
    b_d = nc.dram_tensor("bt", [CIN, 2], F32, kind="ExternalInput")
    y_d = nc.dram_tensor("y", [BPC, COUT, H, W], F32, kind="ExternalOutput")

    with tile.TileContext(nc) as tc:
        with (
            tc.tile_pool(name="xpool", bufs=BPC) as xpool,
            tc.tile_pool(name="wpool", bufs=1) as wpool,
            tc.tile_pool(name="bpool", bufs=1) as bpool,
            tc.tile_pool(name="warms", bufs=1) as warms,
            tc.tile_pool(name="psum", bufs=7, space="PSUM") as psum,
            tc.tile_pool(name="warmp", bufs=1, space="PSUM") as warmp,
            tc.tile_pool(name="opool", bufs=4) as opool,
        ):
            # --- PE warmup: keep the HAM activity window busy while the
            # first input chunks stream in, so real matmuls run at 2.4GHz.
            wsrc = warms.tile([128, 128], F32)
            nc.gpsimd.memset(wsrc[:], 0.0)
            wps = warmp.tile([128, 64], F32)
            for _ in range(NWARM):
                nc.tensor.matmul(wps[:], wsrc[:].bitcast(F32R),
                                 wsrc[:, 0:64].bitcast(F32R),
                                 start=True, stop=True)

            # --- loads (sync engine = one HWDGE FIFO ring, program order):
            # interleave so the first psum group's deps (chunk0 + w-half-0)
            # land first on the FIFO, then the rest.
            x_ts = []
            x_t0 = xpool.tile([CIN, HP, WP], F32R, tag="x")
            x_ts.append(x_t0)
            w_t = wpool.tile([CIN, 2 * 9 * 128], F32R)
            b_t = bpool.tile([CIN, 2], F32)

            def xload(x_t, img, c):
                a, b = XCHUNKS[c]
                nc.sync.dma_start(x_t[:, a:b, :], x_d[img, :, a:b, :])

            # x chunks on the sync HWDGE ring; weights/bias on the scalar
            # engine's separate HWDGE ring so they transfer in parallel.
            nc.scalar.dma_start(w_t[:, :9 * 128], w_d[:, :9 * 128])
            nc.scalar.dma_start(w_t[:, 9 * 128:], w_d[:, 9 * 128:])
            nc.scalar.dma_start(b_t[:], b_d[:])
            for c in range(len(XCHUNKS)):
                xload(x_t0, 0, c)
            x_t1 = xpool.tile([CIN, HP, WP], F32R, tag="x")
            x_ts.append(x_t1)

            def img1_load(c):
                xload(x_t1, 1, c)

            def emit_chunk(img, cb, c0, nrows):
                x_t = x_ts[img]
                ot = opool.tile([128, NCHUNK, W], F32, tag="o")
                for r0 in range(c0, c0 + nrows, NR):
                    ps = psum.tile([128, NR, W], F32, tag="ps")
                    k = 0
                    for dy in range(KH):
                        for dx in range(KW):
                            idx = (cb * 3 + dy) * 3 + dx
                            nc.tensor.matmul(
                                ps[:],
                                w_t[:, idx * 128:(idx + 1) * 128],
                                x_t[:, r0 + dy:r0 + dy + NR, dx:dx + W],
                                start=(k == 0),
                                stop=(k == 8),
                            )
                            k += 1
                    nc.scalar.activation(
                        ot[:, r0 - c0:r0 - c0 + NR, :],
                        ps[:],
                        mybir.ActivationFunctionType.Identity,
                        bias=b_t[:, cb:cb + 1],
                    )
                nc.gpsimd.dma_start(
                    y_d[img, cb * 128:(cb + 1) * 128, c0:c0 + nrows, :],
                    ot[:, :nrows, :],
                )

            for img in range(BPC):
                for cb in range(2):
                    last = img == BPC - 1 and cb == 1
                    for ci, c0 in enumerate(range(0, H, NCHUNK)):
                        if last and c0 + NCHUNK >= H:
                            # split the final chunk for a shorter DMA tail
                            emit_chunk(img, cb, c0, NCHUNK // 2)
                            emit_chunk(img, cb, c0 + NCHUNK // 2, NCHUNK // 2)
                        else:
                            emit_chunk(img, cb, c0, NCHUNK)
                        # defer image-1 chunk loads into image-0/cb0 compute
                        if img == 0 and cb == 0 and ci < len(XCHUNKS) - 1:
                            img1_load(ci)
                    if img == 0 and cb == 0:
                        img1_load(7)
    nc.compile()
    return nc


def _prep(x, weight, bias):
    x = np.asarray(x, dtype=np.float32)
    weight = np.asarray(weight, dtype=np.float32)
    bias = np.asarray(bias, dtype=np.float32)
    xp = np.pad(x, ((0, 0), (0, 0), (1, 1), (1, 1)))
    # wt[cin, ((cb*3+dy)*3+dx)*128 + co] = weight[cb*128+co, cin, dy, dx]
    wt = np.ascontiguousarray(
        weight.reshape(2, 128, CIN, KH, KW).transpose(2, 0, 3, 4, 1).reshape(CIN, -1)
    )
    bt = np.ascontiguousarray(bias.reshape(2, 128).T)
    in_maps = [
        {
            "xp": np.ascontiguousarray(xp[c * BPC:(c + 1) * BPC]),
            "wt": wt,
            "bt": bt,
        }
        for c in range(NCORES)
    ]
    return in_maps


def _run(x, weight, bias, **spmd_kwargs):
    if "nc" not in _cache:
        _cache["nc"] = _build()
    nc = _cache["nc"]
    in_maps = _prep(x, weight, bias)
    res = run_bass_kernel_spmd(nc, in_maps, list(range(NCORES)), **spmd_kwargs)
    y = np.concatenate([res.results[c]["y"] for c in range(NCORES)], axis=0)
    return y, res


def kernel(x, weight, bias):
    y, _ = _run(x, weight, bias)
    return y
